# revision 25
# baseline (speedup 1.0000x reference)
"""Trainium2 Bass kernel for FFSpikingLayer (Linear [F->H] + multistep LIF).

Math (per core shard):
    cur[t,b,h] = sum_f x[t,b,f] * W[h,f] + bias[h]
    v  = v + (cur - v)/2 ;  spk = (v >= 1) ;  v = v*(1-spk)      (T steps)
    out: spk_seq [T,B,H], count[b,h] = sum_t spk

Distribution: 8 cores = 2-way shard over B x 4-way shard over H.
Each core: B_local=128, H_local=1024, full T=32, full F=2048.

GEMM: fp16 hi/lo split, 3 terms (hh + hl + lh), inputs pre-scaled by
S=2^11 so the lo parts stay in fp16 normal range. PE runs fp16 at
1 cycle/row (4x faster than native fp32 matmul); dropped lo*lo term
contributes ~1e-7 relative error. All work in u = 2*S^2*v space:
  u_t = 0.5*r_{t-1} + curS_t ; spk = (u >= 2*S^2) ; r_t = (u < 2*S^2)*u
which matches the reference scan to ~1 ulp (power-of-2 scalings exact).

The LIF scan runs on VectorE in [b=128 partitions, h=1024 free] tiles,
which is exactly the matmul output layout -> no on-device transposes.
Host pre-transposes x/W into PE-friendly layouts (free: not device time).
"""

import numpy as np

T, B, F, H = 32, 256, 2048, 4096
NB, NH = 2, 4  # core grid: 2 B-shards x 4 H-shards = 8 cores
BL, HL = B // NB, H // NH  # 128, 1024
KT = F // 128  # 16 contraction k-tiles
NSPLIT = HL // 512  # psum column halves (matmul N<=512 fp32-out)
S = np.float32(2.0**11)  # fp16 split scale
THRESH = float(2.0 * S * S)  # spike threshold in u-space (2*2^22 = 2^23)

_CACHED = {}


def _build_bass(t_=T, kt_=KT, bl_=BL, hl_=HL, with_bias=True):
    import concourse.bass as bass
    import concourse.tile as tile
    from concourse import bacc, mybir
    from contextlib import ExitStack

    f16 = mybir.dt.float16
    f32 = mybir.dt.float32
    AO = mybir.AluOpType

    nc = bacc.Bacc(trn_type="TRN2", debug=False)

    # Host-prepared layouts:
    #   xh/xl[t, p, k*bl + b] = fp16 split of S * x[t, b, k*128+p]
    #   wh/wl[p, k*hl + h]    = fp16 split of S * W[h0+h, k*128+p]
    #   bias2[0, h] = fp16(b[h]*S); bias2[1, h] = fp16((b[h]*S - hi)*S)
    xh = nc.dram_tensor("xh", [t_, 128, kt_ * bl_], f16, kind="ExternalInput")
    xl = nc.dram_tensor("xl", [t_, 128, kt_ * bl_], f16, kind="ExternalInput")
    wh = nc.dram_tensor("wh", [128, kt_ * hl_], f16, kind="ExternalInput")
    wl = nc.dram_tensor("wl", [128, kt_ * hl_], f16, kind="ExternalInput")
    if with_bias:
        bias2 = nc.dram_tensor("bias2", [2, hl_], f16, kind="ExternalInput")
        bxc = nc.dram_tensor("bxc", [2, bl_], f16, kind="ExternalInput")
    spk = nc.dram_tensor("spk", [t_, bl_, hl_], f32, kind="ExternalOutput")
    cnt = nc.dram_tensor("cnt", [bl_, hl_], f32, kind="ExternalOutput")

    with tile.TileContext(nc) as tc, ExitStack() as ctx:
        wpool = ctx.enter_context(tc.tile_pool(name="w", bufs=1))
        xpool = ctx.enter_context(tc.tile_pool(name="x", bufs=4))
        pspool = ctx.enter_context(tc.tile_pool(name="ps", bufs=4, space="PSUM"))
        upool = ctx.enter_context(tc.tile_pool(name="u", bufs=2))
        rpool = ctx.enter_context(tc.tile_pool(name="r", bufs=2))
        spool = ctx.enter_context(tc.tile_pool(name="s", bufs=4))
        misc = ctx.enter_context(tc.tile_pool(name="mi", bufs=1))

        if with_bias:
            bias_sb = misc.tile([2, hl_], f16, name="bias_sb")
            nc.gpsimd.dma_start(bias_sb[:], bias2.ap())
            # bias stationary rows: [S, 1] so S*row0 + 1*row1 = S^2*b
            bx = misc.tile([2, bl_], f16, name="bx")
            nc.gpsimd.dma_start(bx[:], bxc.ap())

        # W loads split per k-tile chunk, in first-use order (all hi, then
        # all lo), so the first matmul only waits on one 512KB chunk, not 8MB.
        wh_sb = wpool.tile([128, kt_ * hl_], f16, name="wh_sb")
        wl_sb = wpool.tile([128, kt_ * hl_], f16, name="wl_sb")
        cw = max(1, kt_ // 8)
        wh_chunks = [(0, 1), (1, 2)] if kt_ > 2 else []
        wh_chunks += [(k0, k0 + cw) for k0 in range(2 if kt_ > 2 else 0, kt_, cw)]
        for k0, k1 in wh_chunks:
            nc.sync.dma_start(
                wh_sb[:, k0 * hl_ : k1 * hl_],
                wh.ap()[:, k0 * hl_ : k1 * hl_],
            )
        cw2 = max(1, kt_ // 4)
        for k0 in range(0, kt_, cw2):
            nc.sync.dma_start(
                wl_sb[:, k0 * hl_ : (k0 + cw2) * hl_],
                wl.ap()[:, k0 * hl_ : (k0 + cw2) * hl_],
            )

        zero = misc.tile([128, hl_], f32, name="zero")
        nc.vector.memset(zero[:], 0.0)
        cnt_sb = misc.tile([128, hl_], f32, name="cnt_sb")
        nc.vector.memset(cnt_sb[:], 0.0)

        r_prev = zero
        for t in range(t_):
            xh_t = xpool.tile([128, kt_ * bl_], f16, name="xh_t", tag="xh_t")
            if t == 0 and kt_ > 2:
                # split the very first load so MM 0 gates on a 64KB chunk
                c0 = 2 * bl_
                nc.scalar.dma_start(xh_t[:, :c0], xh.ap()[t][:, :c0])
                nc.scalar.dma_start(xh_t[:, c0:], xh.ap()[t][:, c0:])
            else:
                nc.scalar.dma_start(xh_t[:], xh.ap()[t])
            xl_t = xpool.tile([128, kt_ * bl_], f16, name="xl_t", tag="xl_t")
            nc.scalar.dma_start(xl_t[:], xl.ap()[t])

            # loop (term, k, half): consecutive matmuls share the stationary
            # x k-tile and W is consumed at half the early-warmup rate.
            # Term order hh, lh, hl: both wh-terms run before any wl-term, so
            # at t=0 the wl DMA stream has ~7us more headroom to arrive.
            ps_t = pspool.tile([128, hl_], f32, name="ps_t", tag="ps_t")
            nhalf = hl_ // 512
            nmm = 3 * kt_ * nhalf
            for ti, (xs, ws) in enumerate(
                ((xh_t, wh_sb), (xl_t, wh_sb), (xh_t, wl_sb))
            ):
                for k in range(kt_):
                    for half in range(nhalf):
                        cs = slice(half * 512, half * 512 + 512)
                        nmm -= 1
                        nc.tensor.matmul(
                            ps_t[:, cs],
                            xs[:, k * bl_ : (k + 1) * bl_],
                            ws[:, k * hl_ + half * 512 : k * hl_ + half * 512 + 512],
                            start=(ti == 0 and k == 0),
                            stop=(not with_bias) and nmm < nhalf,
                        )
            if with_bias:
                for half in range(nhalf):
                    cs = slice(half * 512, half * 512 + 512)
                    nc.tensor.matmul(
                        ps_t[:, cs], bx[:, :], bias_sb[:, cs], start=False, stop=True
                    )

            if t < t_ - 1:
                # u = 0.5*r_prev + cur
                u_t = upool.tile([128, hl_], f32, name="u_t", tag="u_t")
                nc.vector.scalar_tensor_tensor(
                    u_t[:], r_prev[:], 0.5, ps_t[:], op0=AO.mult, op1=AO.add
                )
                # spk = (u >= 2*S^2) as 1.0/0.0
                spk_t = spool.tile([128, hl_], f32, name="spk_t", tag="spk_t")
                nc.vector.tensor_scalar(
                    spk_t[:], u_t[:], THRESH, None, op0=AO.is_ge
                )
                nc.sync.dma_start(spk.ap()[t], spk_t[:])
                # cnt += spk
                nc.vector.tensor_tensor(cnt_sb[:], cnt_sb[:], spk_t[:], op=AO.add)
                # r = (u < thresh) * u   (post-reset, pre-halved state)
                r_t = rpool.tile([128, hl_], f32, name="r_t", tag="r_t")
                nc.vector.scalar_tensor_tensor(
                    r_t[:], u_t[:], THRESH, u_t[:], op0=AO.is_lt, op1=AO.mult
                )
                r_prev = r_t
            else:
                # last step: per-half scan so the spk/cnt stores stream out
                # while the other half computes; dead r-update skipped.
                u_t = upool.tile([128, hl_], f32, name="u_t", tag="u_t")
                spk_t = spool.tile([128, hl_], f32, name="spk_t", tag="spk_t")
                for half in range(hl_ // 512):
                    cs = slice(half * 512, half * 512 + 512)
                    nc.vector.scalar_tensor_tensor(
                        u_t[:, cs], r_prev[:, cs], 0.5, ps_t[:, cs],
                        op0=AO.mult, op1=AO.add,
                    )
                    nc.vector.tensor_scalar(
                        spk_t[:, cs], u_t[:, cs], THRESH, None, op0=AO.is_ge
                    )
                    nc.sync.dma_start(spk.ap()[t][:, cs], spk_t[:, cs])
                    nc.vector.tensor_tensor(
                        cnt_sb[:, cs], cnt_sb[:, cs], spk_t[:, cs], op=AO.add
                    )
                    nc.scalar.dma_start(cnt.ap()[:, cs], cnt_sb[:, cs])

    nc.compile()
    return nc


def _split_f16(a):
    hi = a.astype(np.float16)
    lo = (a - hi.astype(np.float32)).astype(np.float16)
    return hi, lo


def _prep_inputs(x_seq, W, b):
    """Host-side shard + transpose + fp16 split. Returns in_maps for 8 cores."""
    x_seq = np.asarray(x_seq, dtype=np.float32)
    W = np.asarray(W, dtype=np.float32)
    b = np.asarray(b, dtype=np.float32)

    xmaps = []
    for g in range(NB):
        xg = x_seq[:, g * BL : (g + 1) * BL, :]  # [T, BL, F]
        xt = np.ascontiguousarray(xg.transpose(0, 2, 1)) * S  # [T, F, BL]
        # [T, F, BL] -> [T, KT, 128, BL] -> [T, 128, KT, BL]
        xt = np.ascontiguousarray(
            xt.reshape(T, KT, 128, BL).transpose(0, 2, 1, 3)
        ).reshape(T, 128, KT * BL)
        xmaps.append(_split_f16(xt))

    wmaps = []
    bmaps = []
    for j in range(NH):
        wj = W[j * HL : (j + 1) * HL, :]  # [HL, F]
        wt = np.ascontiguousarray(wj.T) * S  # [F, HL]
        wt = np.ascontiguousarray(
            wt.reshape(KT, 128, HL).transpose(1, 0, 2)
        ).reshape(128, KT * HL)
        wmaps.append(_split_f16(wt))
        bs = b[j * HL : (j + 1) * HL] * S
        b_hi = bs.astype(np.float16)
        b_lo = ((bs - b_hi.astype(np.float32)) * S).astype(np.float16)
        bmaps.append(np.stack([b_hi, b_lo]))

    with_bias = bool(np.any(b))
    in_maps = []
    for c in range(NB * NH):
        bg, hg = c // NH, c % NH
        m = {
            "xh": xmaps[bg][0],
            "xl": xmaps[bg][1],
            "wh": wmaps[hg][0],
            "wl": wmaps[hg][1],
        }
        if with_bias:
            m["bias2"] = bmaps[hg]
            m["bxc"] = bx_const()
        in_maps.append(m)
    return in_maps, with_bias


def bx_const(bl_=BL):
    out = np.empty((2, bl_), dtype=np.float16)
    out[0, :] = S
    out[1, :] = 1.0
    return out


def _run(in_maps, with_bias, trace=False):
    from concourse.bass_utils import run_bass_kernel_spmd

    key = ("nc", with_bias)
    if key not in _CACHED:
        _CACHED[key] = _build_bass(with_bias=with_bias)
    res = run_bass_kernel_spmd(
        _CACHED[key], in_maps, core_ids=list(range(NB * NH)), trace=trace
    )
    return res


def _assemble(results):
    spk_full = np.empty((T, B, H), dtype=np.float32)
    cnt_full = np.empty((B, H), dtype=np.float32)
    for c in range(NB * NH):
        bg, hg = c // NH, c % NH
        bsl = slice(bg * BL, (bg + 1) * BL)
        hsl = slice(hg * HL, (hg + 1) * HL)
        spk_full[:, bsl, hsl] = results[c]["spk"]
        cnt_full[bsl, hsl] = results[c]["cnt"]
    return spk_full, cnt_full


def kernel(x_seq, W, b):
    in_maps, with_bias = _prep_inputs(x_seq, W, b)
    res = _run(in_maps, with_bias, trace=False)
    return _assemble(res.results)


# revision 26
# speedup vs baseline: 1.0036x; 1.0036x over previous
"""Trainium2 Bass kernel for FFSpikingLayer (Linear [F->H] + multistep LIF).

Math (per core shard):
    cur[t,b,h] = sum_f x[t,b,f] * W[h,f] + bias[h]
    v  = v + (cur - v)/2 ;  spk = (v >= 1) ;  v = v*(1-spk)      (T steps)
    out: spk_seq [T,B,H], count[b,h] = sum_t spk

Distribution: 8 cores = 2-way shard over B x 4-way shard over H.
Each core: B_local=128, H_local=1024, full T=32, full F=2048.

GEMM: fp16 hi/lo split, 3 terms (hh + hl + lh), inputs pre-scaled by
S=2^11 so the lo parts stay in fp16 normal range. PE runs fp16 at
1 cycle/row (4x faster than native fp32 matmul); dropped lo*lo term
contributes ~1e-7 relative error. All work in u = 2*S^2*v space:
  u_t = 0.5*r_{t-1} + curS_t ; spk = (u >= 2*S^2) ; r_t = (u < 2*S^2)*u
which matches the reference scan to ~1 ulp (power-of-2 scalings exact).

The LIF scan runs on VectorE in [b=128 partitions, h=1024 free] tiles,
which is exactly the matmul output layout -> no on-device transposes.
Host pre-transposes x/W into PE-friendly layouts (free: not device time).
"""

import numpy as np

T, B, F, H = 32, 256, 2048, 4096
NB, NH = 2, 4  # core grid: 2 B-shards x 4 H-shards = 8 cores
BL, HL = B // NB, H // NH  # 128, 1024
KT = F // 128  # 16 contraction k-tiles
NSPLIT = HL // 512  # psum column halves (matmul N<=512 fp32-out)
S = np.float32(2.0**11)  # fp16 split scale
THRESH = float(2.0 * S * S)  # spike threshold in u-space (2*2^22 = 2^23)

_CACHED = {}


def _build_bass(t_=T, kt_=KT, bl_=BL, hl_=HL, with_bias=True):
    import concourse.bass as bass
    import concourse.tile as tile
    from concourse import bacc, mybir
    from contextlib import ExitStack

    f16 = mybir.dt.float16
    f32 = mybir.dt.float32
    AO = mybir.AluOpType

    nc = bacc.Bacc(trn_type="TRN2", debug=False)

    # Host-prepared layouts:
    #   xh/xl[t, p, k*bl + b] = fp16 split of S * x[t, b, k*128+p]
    #   wh/wl[p, k*hl + h]    = fp16 split of S * W[h0+h, k*128+p]
    #   bias2[0, h] = fp16(b[h]*S); bias2[1, h] = fp16((b[h]*S - hi)*S)
    xh = nc.dram_tensor("xh", [t_, 128, kt_ * bl_], f16, kind="ExternalInput")
    xl = nc.dram_tensor("xl", [t_, 128, kt_ * bl_], f16, kind="ExternalInput")
    wh = nc.dram_tensor("wh", [128, kt_ * hl_], f16, kind="ExternalInput")
    wl = nc.dram_tensor("wl", [128, kt_ * hl_], f16, kind="ExternalInput")
    if with_bias:
        bias2 = nc.dram_tensor("bias2", [2, hl_], f16, kind="ExternalInput")
        bxc = nc.dram_tensor("bxc", [2, bl_], f16, kind="ExternalInput")
    spk = nc.dram_tensor("spk", [t_, bl_, hl_], f32, kind="ExternalOutput")
    cnt = nc.dram_tensor("cnt", [bl_, hl_], f32, kind="ExternalOutput")

    with tile.TileContext(nc) as tc, ExitStack() as ctx:
        wpool = ctx.enter_context(tc.tile_pool(name="w", bufs=1))
        xpool = ctx.enter_context(tc.tile_pool(name="x", bufs=4))
        pspool = ctx.enter_context(tc.tile_pool(name="ps", bufs=4, space="PSUM"))
        upool = ctx.enter_context(tc.tile_pool(name="u", bufs=2))
        rpool = ctx.enter_context(tc.tile_pool(name="r", bufs=2))
        spool = ctx.enter_context(tc.tile_pool(name="s", bufs=4))
        misc = ctx.enter_context(tc.tile_pool(name="mi", bufs=1))

        if with_bias:
            bias_sb = misc.tile([2, hl_], f16, name="bias_sb")
            nc.gpsimd.dma_start(bias_sb[:], bias2.ap())
            # bias stationary rows: [S, 1] so S*row0 + 1*row1 = S^2*b
            bx = misc.tile([2, bl_], f16, name="bx")
            nc.gpsimd.dma_start(bx[:], bxc.ap())

        # W loads split per k-tile chunk, in first-use order (all hi, then
        # all lo), so the first matmul only waits on one 512KB chunk, not 8MB.
        wh_sb = wpool.tile([128, kt_ * hl_], f16, name="wh_sb")
        wl_sb = wpool.tile([128, kt_ * hl_], f16, name="wl_sb")
        cw = max(1, kt_ // 8)
        for k0 in range(0, kt_, cw):
            nc.sync.dma_start(
                wh_sb[:, k0 * hl_ : (k0 + cw) * hl_],
                wh.ap()[:, k0 * hl_ : (k0 + cw) * hl_],
            )
        cw2 = max(1, kt_ // 4)
        for k0 in range(0, kt_, cw2):
            nc.sync.dma_start(
                wl_sb[:, k0 * hl_ : (k0 + cw2) * hl_],
                wl.ap()[:, k0 * hl_ : (k0 + cw2) * hl_],
            )

        zero = misc.tile([128, hl_], f32, name="zero")
        nc.vector.memset(zero[:], 0.0)
        cnt_sb = misc.tile([128, hl_], f32, name="cnt_sb")
        nc.vector.memset(cnt_sb[:], 0.0)

        r_prev = zero
        for t in range(t_):
            xh_t = xpool.tile([128, kt_ * bl_], f16, name="xh_t", tag="xh_t")
            if t == 0 and kt_ > 2:
                # split the very first load so MM 0 gates on a 64KB chunk
                c0 = 2 * bl_
                nc.scalar.dma_start(xh_t[:, :c0], xh.ap()[t][:, :c0])
                nc.scalar.dma_start(xh_t[:, c0:], xh.ap()[t][:, c0:])
            else:
                nc.scalar.dma_start(xh_t[:], xh.ap()[t])
            xl_t = xpool.tile([128, kt_ * bl_], f16, name="xl_t", tag="xl_t")
            nc.scalar.dma_start(xl_t[:], xl.ap()[t])

            # loop (term, k, half): consecutive matmuls share the stationary
            # x k-tile and W is consumed at half the early-warmup rate.
            # Term order hh, lh, hl: both wh-terms run before any wl-term, so
            # at t=0 the wl DMA stream has ~7us more headroom to arrive.
            ps_t = pspool.tile([128, hl_], f32, name="ps_t", tag="ps_t")
            nhalf = hl_ // 512
            nmm = 3 * kt_ * nhalf
            for ti, (xs, ws) in enumerate(
                ((xh_t, wh_sb), (xl_t, wh_sb), (xh_t, wl_sb))
            ):
                for k in range(kt_):
                    for half in range(nhalf):
                        cs = slice(half * 512, half * 512 + 512)
                        nmm -= 1
                        nc.tensor.matmul(
                            ps_t[:, cs],
                            xs[:, k * bl_ : (k + 1) * bl_],
                            ws[:, k * hl_ + half * 512 : k * hl_ + half * 512 + 512],
                            start=(ti == 0 and k == 0),
                            stop=(not with_bias) and nmm < nhalf,
                        )
            if with_bias:
                for half in range(nhalf):
                    cs = slice(half * 512, half * 512 + 512)
                    nc.tensor.matmul(
                        ps_t[:, cs], bx[:, :], bias_sb[:, cs], start=False, stop=True
                    )

            # u = 0.5*r_prev + cur
            u_t = upool.tile([128, hl_], f32, name="u_t", tag="u_t")
            nc.vector.scalar_tensor_tensor(
                u_t[:], r_prev[:], 0.5, ps_t[:], op0=AO.mult, op1=AO.add
            )
            # spk = (u >= 2*S^2) as 1.0/0.0
            spk_t = spool.tile([128, hl_], f32, name="spk_t", tag="spk_t")
            nc.vector.tensor_scalar(
                spk_t[:], u_t[:], THRESH, None, op0=AO.is_ge
            )
            nc.sync.dma_start(spk.ap()[t], spk_t[:])
            # cnt += spk
            nc.vector.tensor_tensor(cnt_sb[:], cnt_sb[:], spk_t[:], op=AO.add)
            if t < t_ - 1:
                # r = (u < thresh) * u   (post-reset, pre-halved state);
                # dead after the last step, so skip it there (shorter tail)
                r_t = rpool.tile([128, hl_], f32, name="r_t", tag="r_t")
                nc.vector.scalar_tensor_tensor(
                    r_t[:], u_t[:], THRESH, u_t[:], op0=AO.is_lt, op1=AO.mult
                )
                r_prev = r_t

        nc.sync.dma_start(cnt.ap(), cnt_sb[:])

    nc.compile()
    return nc


def _split_f16(a):
    hi = a.astype(np.float16)
    lo = (a - hi.astype(np.float32)).astype(np.float16)
    return hi, lo


def _prep_inputs(x_seq, W, b):
    """Host-side shard + transpose + fp16 split. Returns in_maps for 8 cores."""
    x_seq = np.asarray(x_seq, dtype=np.float32)
    W = np.asarray(W, dtype=np.float32)
    b = np.asarray(b, dtype=np.float32)

    xmaps = []
    for g in range(NB):
        xg = x_seq[:, g * BL : (g + 1) * BL, :]  # [T, BL, F]
        xt = np.ascontiguousarray(xg.transpose(0, 2, 1)) * S  # [T, F, BL]
        # [T, F, BL] -> [T, KT, 128, BL] -> [T, 128, KT, BL]
        xt = np.ascontiguousarray(
            xt.reshape(T, KT, 128, BL).transpose(0, 2, 1, 3)
        ).reshape(T, 128, KT * BL)
        xmaps.append(_split_f16(xt))

    wmaps = []
    bmaps = []
    for j in range(NH):
        wj = W[j * HL : (j + 1) * HL, :]  # [HL, F]
        wt = np.ascontiguousarray(wj.T) * S  # [F, HL]
        wt = np.ascontiguousarray(
            wt.reshape(KT, 128, HL).transpose(1, 0, 2)
        ).reshape(128, KT * HL)
        wmaps.append(_split_f16(wt))
        bs = b[j * HL : (j + 1) * HL] * S
        b_hi = bs.astype(np.float16)
        b_lo = ((bs - b_hi.astype(np.float32)) * S).astype(np.float16)
        bmaps.append(np.stack([b_hi, b_lo]))

    with_bias = bool(np.any(b))
    in_maps = []
    for c in range(NB * NH):
        bg, hg = c // NH, c % NH
        m = {
            "xh": xmaps[bg][0],
            "xl": xmaps[bg][1],
            "wh": wmaps[hg][0],
            "wl": wmaps[hg][1],
        }
        if with_bias:
            m["bias2"] = bmaps[hg]
            m["bxc"] = bx_const()
        in_maps.append(m)
    return in_maps, with_bias


def bx_const(bl_=BL):
    out = np.empty((2, bl_), dtype=np.float16)
    out[0, :] = S
    out[1, :] = 1.0
    return out


def _run(in_maps, with_bias, trace=False):
    from concourse.bass_utils import run_bass_kernel_spmd

    key = ("nc", with_bias)
    if key not in _CACHED:
        _CACHED[key] = _build_bass(with_bias=with_bias)
    res = run_bass_kernel_spmd(
        _CACHED[key], in_maps, core_ids=list(range(NB * NH)), trace=trace
    )
    return res


def _assemble(results):
    spk_full = np.empty((T, B, H), dtype=np.float32)
    cnt_full = np.empty((B, H), dtype=np.float32)
    for c in range(NB * NH):
        bg, hg = c // NH, c % NH
        bsl = slice(bg * BL, (bg + 1) * BL)
        hsl = slice(hg * HL, (hg + 1) * HL)
        spk_full[:, bsl, hsl] = results[c]["spk"]
        cnt_full[bsl, hsl] = results[c]["cnt"]
    return spk_full, cnt_full


def kernel(x_seq, W, b):
    in_maps, with_bias = _prep_inputs(x_seq, W, b)
    res = _run(in_maps, with_bias, trace=False)
    return _assemble(res.results)


# revision 27
# speedup vs baseline: 1.0114x; 1.0078x over previous
"""Trainium2 Bass kernel for FFSpikingLayer (Linear [F->H] + multistep LIF).

Math (per core shard):
    cur[t,b,h] = sum_f x[t,b,f] * W[h,f] + bias[h]
    v  = v + (cur - v)/2 ;  spk = (v >= 1) ;  v = v*(1-spk)      (T steps)
    out: spk_seq [T,B,H], count[b,h] = sum_t spk

Distribution: 8 cores = 2-way shard over B x 4-way shard over H
(B_local=128, H_local=1024 per core) -- measured better than 1x8
(H-only) because it minimizes total per-core DMA bytes; the startup
window is HBM-bandwidth-bound.

GEMM: fp16 hi/lo split, 3 terms (x_hi@w_hi + x_lo@w_hi + x_hi@w_lo),
inputs pre-scaled by S=2^11 so the lo parts stay in fp16 normal range.
PE runs fp16 at 1 cycle/row; 3 passes beat native fp32 (4 cycles/row)
by 1.33x and reproduce the CPU fp32 reference bit-exactly here
(dropped lo*lo term ~1e-7 relative). Alternatives measured and
rejected on HW: fp32r keeps only ~12.5 mantissa bits; bf16 moving
operands stream at the same 216ns/MM as fp16 (no 2x mode); fp8
DoubleRow digit schemes never reach the needed ~18-bit product
precision at fewer total cycles. Term order: both w_hi-consuming
terms run first so the w_lo DMA stream has more time to arrive.

The LIF scan runs on VectorE in u = 2*S^2*v space:
  u_t = 0.5*r_{t-1} + curS_t ; spk = (u >= 2*S^2) ; r_t = (u < 2*S^2)*u
(power-of-2 scalings exact; matches the reference scan to ~1 ulp), in
[b=128 partitions, h=1024 free] tiles -- exactly the matmul output
layout, so there are no on-device transposes. Host pre-transposes x/W
into PE-friendly layouts (host prep is not device time).

Measured: ~691-698us HW exec per run (TensorE 96% busy at the 216ns
N=512 issue floor; ~27us fixed preamble/lead-in/drain overhead),
relative error 0.0 vs the jax CPU reference (0/33.5M spike flips).
"""

import numpy as np

T, B, F, H = 32, 256, 2048, 4096
NB, NH = 2, 4  # core grid: 2 B-shards x 4 H-shards = 8 cores
BL, HL = B // NB, H // NH  # 128, 1024
KT = F // 128  # 16 contraction k-tiles
NSPLIT = HL // 512  # psum column halves (matmul N<=512 fp32-out)
S = np.float32(2.0**11)  # fp16 split scale
THRESH = float(2.0 * S * S)  # spike threshold in u-space (2*2^22 = 2^23)

_CACHED = {}


def _build_bass(t_=T, kt_=KT, bl_=BL, hl_=HL, with_bias=True):
    import concourse.bass as bass
    import concourse.tile as tile
    from concourse import bacc, mybir
    from contextlib import ExitStack

    f16 = mybir.dt.float16
    f32 = mybir.dt.float32
    AO = mybir.AluOpType

    nc = bacc.Bacc(trn_type="TRN2", debug=False)

    # Host-prepared layouts:
    #   xh/xl[t, p, k*bl + b] = fp16 split of S * x[t, b, k*128+p]
    #   wh/wl[p, k*hl + h]    = fp16 split of S * W[h0+h, k*128+p]
    #   bias2[0, h] = fp16(b[h]*S); bias2[1, h] = fp16((b[h]*S - hi)*S)
    xh = nc.dram_tensor("xh", [t_, 128, kt_ * bl_], f16, kind="ExternalInput")
    xl = nc.dram_tensor("xl", [t_, 128, kt_ * bl_], f16, kind="ExternalInput")
    wh = nc.dram_tensor("wh", [128, kt_ * hl_], f16, kind="ExternalInput")
    wl = nc.dram_tensor("wl", [128, kt_ * hl_], f16, kind="ExternalInput")
    if with_bias:
        bias2 = nc.dram_tensor("bias2", [2, hl_], f16, kind="ExternalInput")
        bxc = nc.dram_tensor("bxc", [2, bl_], f16, kind="ExternalInput")
    spk = nc.dram_tensor("spk", [t_, bl_, hl_], f32, kind="ExternalOutput")
    cnt = nc.dram_tensor("cnt", [bl_, hl_], f32, kind="ExternalOutput")

    with tile.TileContext(nc) as tc, ExitStack() as ctx:
        wpool = ctx.enter_context(tc.tile_pool(name="w", bufs=1))
        xpool = ctx.enter_context(tc.tile_pool(name="x", bufs=4))
        pspool = ctx.enter_context(tc.tile_pool(name="ps", bufs=4, space="PSUM"))
        upool = ctx.enter_context(tc.tile_pool(name="u", bufs=2))
        rpool = ctx.enter_context(tc.tile_pool(name="r", bufs=2))
        spool = ctx.enter_context(tc.tile_pool(name="s", bufs=4))
        misc = ctx.enter_context(tc.tile_pool(name="mi", bufs=1))

        if with_bias:
            bias_sb = misc.tile([2, hl_], f16, name="bias_sb")
            nc.gpsimd.dma_start(bias_sb[:], bias2.ap())
            # bias stationary rows: [S, 1] so S*row0 + 1*row1 = S^2*b
            bx = misc.tile([2, bl_], f16, name="bx")
            nc.gpsimd.dma_start(bx[:], bxc.ap())

        # W loads split per k-tile chunk, in first-use order (all hi, then
        # all lo), so the first matmul only waits on one 512KB chunk, not 8MB.
        wh_sb = wpool.tile([128, kt_ * hl_], f16, name="wh_sb")
        wl_sb = wpool.tile([128, kt_ * hl_], f16, name="wl_sb")
        cw = max(1, kt_ // 8)
        for k0 in range(0, kt_, cw):
            nc.sync.dma_start(
                wh_sb[:, k0 * hl_ : (k0 + cw) * hl_],
                wh.ap()[:, k0 * hl_ : (k0 + cw) * hl_],
            )
        cw2 = max(1, kt_ // 4)
        for k0 in range(0, kt_, cw2):
            nc.sync.dma_start(
                wl_sb[:, k0 * hl_ : (k0 + cw2) * hl_],
                wl.ap()[:, k0 * hl_ : (k0 + cw2) * hl_],
            )

        zero = misc.tile([128, hl_], f32, name="zero")
        nc.vector.memset(zero[:], 0.0)
        cnt_sb = misc.tile([128, hl_], f32, name="cnt_sb")
        nc.vector.memset(cnt_sb[:], 0.0)

        r_prev = zero
        for t in range(t_):
            xh_t = xpool.tile([128, kt_ * bl_], f16, name="xh_t", tag="xh_t")
            if t == 0 and kt_ > 2:
                # split the very first load so MM 0 gates on a 64KB chunk
                c0 = 2 * bl_
                nc.scalar.dma_start(xh_t[:, :c0], xh.ap()[t][:, :c0])
                nc.scalar.dma_start(xh_t[:, c0:], xh.ap()[t][:, c0:])
            else:
                nc.scalar.dma_start(xh_t[:], xh.ap()[t])
            xl_t = xpool.tile([128, kt_ * bl_], f16, name="xl_t", tag="xl_t")
            nc.scalar.dma_start(xl_t[:], xl.ap()[t])

            # loop (term, k, half): consecutive matmuls share the stationary
            # x k-tile and W is consumed at half the early-warmup rate.
            # Term order hh, lh, hl: both wh-terms run before any wl-term, so
            # at t=0 the wl DMA stream has ~7us more headroom to arrive.
            ps_t = pspool.tile([128, hl_], f32, name="ps_t", tag="ps_t")
            nhalf = hl_ // 512
            nmm = 3 * kt_ * nhalf
            for ti, (xs, ws) in enumerate(
                ((xh_t, wh_sb), (xl_t, wh_sb), (xh_t, wl_sb))
            ):
                for k in range(kt_):
                    for half in range(nhalf):
                        cs = slice(half * 512, half * 512 + 512)
                        nmm -= 1
                        nc.tensor.matmul(
                            ps_t[:, cs],
                            xs[:, k * bl_ : (k + 1) * bl_],
                            ws[:, k * hl_ + half * 512 : k * hl_ + half * 512 + 512],
                            start=(ti == 0 and k == 0),
                            stop=(not with_bias) and nmm < nhalf,
                        )
            if with_bias:
                for half in range(nhalf):
                    cs = slice(half * 512, half * 512 + 512)
                    nc.tensor.matmul(
                        ps_t[:, cs], bx[:, :], bias_sb[:, cs], start=False, stop=True
                    )

            # u = 0.5*r_prev + cur
            u_t = upool.tile([128, hl_], f32, name="u_t", tag="u_t")
            nc.vector.scalar_tensor_tensor(
                u_t[:], r_prev[:], 0.5, ps_t[:], op0=AO.mult, op1=AO.add
            )
            # spk = (u >= 2*S^2) as 1.0/0.0
            spk_t = spool.tile([128, hl_], f32, name="spk_t", tag="spk_t")
            nc.vector.tensor_scalar(
                spk_t[:], u_t[:], THRESH, None, op0=AO.is_ge
            )
            nc.sync.dma_start(spk.ap()[t], spk_t[:])
            # cnt += spk
            nc.vector.tensor_tensor(cnt_sb[:], cnt_sb[:], spk_t[:], op=AO.add)
            if t < t_ - 1:
                # r = (u < thresh) * u   (post-reset, pre-halved state);
                # dead after the last step, so skip it there (shorter tail)
                r_t = rpool.tile([128, hl_], f32, name="r_t", tag="r_t")
                nc.vector.scalar_tensor_tensor(
                    r_t[:], u_t[:], THRESH, u_t[:], op0=AO.is_lt, op1=AO.mult
                )
                r_prev = r_t

        nc.sync.dma_start(cnt.ap(), cnt_sb[:])

    nc.compile()
    return nc


def _split_f16(a):
    hi = a.astype(np.float16)
    lo = (a - hi.astype(np.float32)).astype(np.float16)
    return hi, lo


def _prep_inputs(x_seq, W, b):
    """Host-side shard + transpose + fp16 split. Returns in_maps for 8 cores."""
    x_seq = np.asarray(x_seq, dtype=np.float32)
    W = np.asarray(W, dtype=np.float32)
    b = np.asarray(b, dtype=np.float32)

    xmaps = []
    for g in range(NB):
        xg = x_seq[:, g * BL : (g + 1) * BL, :]  # [T, BL, F]
        xt = np.ascontiguousarray(xg.transpose(0, 2, 1)) * S  # [T, F, BL]
        # [T, F, BL] -> [T, KT, 128, BL] -> [T, 128, KT, BL]
        xt = np.ascontiguousarray(
            xt.reshape(T, KT, 128, BL).transpose(0, 2, 1, 3)
        ).reshape(T, 128, KT * BL)
        xmaps.append(_split_f16(xt))

    wmaps = []
    bmaps = []
    for j in range(NH):
        wj = W[j * HL : (j + 1) * HL, :]  # [HL, F]
        wt = np.ascontiguousarray(wj.T) * S  # [F, HL]
        wt = np.ascontiguousarray(
            wt.reshape(KT, 128, HL).transpose(1, 0, 2)
        ).reshape(128, KT * HL)
        wmaps.append(_split_f16(wt))
        bs = b[j * HL : (j + 1) * HL] * S
        b_hi = bs.astype(np.float16)
        b_lo = ((bs - b_hi.astype(np.float32)) * S).astype(np.float16)
        bmaps.append(np.stack([b_hi, b_lo]))

    with_bias = bool(np.any(b))
    in_maps = []
    for c in range(NB * NH):
        bg, hg = c // NH, c % NH
        m = {
            "xh": xmaps[bg][0],
            "xl": xmaps[bg][1],
            "wh": wmaps[hg][0],
            "wl": wmaps[hg][1],
        }
        if with_bias:
            m["bias2"] = bmaps[hg]
            m["bxc"] = bx_const()
        in_maps.append(m)
    return in_maps, with_bias


def bx_const(bl_=BL):
    out = np.empty((2, bl_), dtype=np.float16)
    out[0, :] = S
    out[1, :] = 1.0
    return out


def _run(in_maps, with_bias, trace=False):
    from concourse.bass_utils import run_bass_kernel_spmd

    key = ("nc", with_bias)
    if key not in _CACHED:
        _CACHED[key] = _build_bass(with_bias=with_bias)
    res = run_bass_kernel_spmd(
        _CACHED[key], in_maps, core_ids=list(range(NB * NH)), trace=trace
    )
    return res


def _assemble(results):
    spk_full = np.empty((T, B, H), dtype=np.float32)
    cnt_full = np.empty((B, H), dtype=np.float32)
    for c in range(NB * NH):
        bg, hg = c // NH, c % NH
        bsl = slice(bg * BL, (bg + 1) * BL)
        hsl = slice(hg * HL, (hg + 1) * HL)
        spk_full[:, bsl, hsl] = results[c]["spk"]
        cnt_full[bsl, hsl] = results[c]["cnt"]
    return spk_full, cnt_full


def kernel(x_seq, W, b):
    in_maps, with_bias = _prep_inputs(x_seq, W, b)
    res = _run(in_maps, with_bias, trace=False)
    return _assemble(res.results)
